# revision 13
# baseline (speedup 1.0000x reference)
"""Multi-head causal attention (B=8, T=2048, C=1024, H=16, D=64) on 8 TRN2 NeuronCores.

Strategy: pure data-parallel over batch (B=8 = n_cores, no collectives).
Each core processes one batch element:
  - transpose x -> xT [C, T] via PE (every C-contraction wants C on partitions
    for both operands)
  - per head-pair g (2 heads packed into 128 partitions):
      QT/KT [128, T] = w_pair.T @ xT     (heads stacked on partition dim)
      V     [s, 8*64] per head-oct       (8 heads packed on free dim, N=512)
      S^T tiles [s 128, tq 512] per head via row-tiled K=64 matmuls
        (tile_position (0,0)/(64,0): the two heads run concurrently on HW)
      P^T = exp(S^T / 32)  (ScalarE; no max-subtraction needed: |logits| < ~1,
        so exp cannot overflow and every row has its diagonal element)
      causal mask via gpsimd affine_select on diagonal tiles only; dead
        columns of diagonal tiles are never computed (lo strip skipping)
      O^T [d, tq] accumulated via col-tiled M=64 matmuls (lhsT = V, also
        concurrent via tile_position (0,0)/(0,64))
      row-sums broadcast to all partitions via ones-matmul (softmax denom),
      reciprocal + multiply folded into the PSUM->SBUF eviction of O^T
  - final projection Y = OT_all.T @ w_proj + bias, contiguous writeback

Matmul dtype: bf16 operands everywhere (USE_BF16=True; fp32r fallback kept).
HW-measured rel err vs float64 oracle: ~3.1e-3 (fp32r path: ~1.8e-3).
"""
import numpy as np

import concourse.bass as bass
import concourse.mybir as mybir
import concourse.tile as tile
from concourse import bacc
from concourse.bass_utils import run_bass_kernel_spmd
from concourse.masks import make_identity

B, T, C = 8, 2048, 1024
H, D = 16, 64
P = 128
KO = C // P          # 8 contraction chunks over C
NT = T // P          # 16 t-tiles of 128
NJ = T // 512        # 4 t-chunks of 512
NPAIR = H // 2       # 8 head pairs
NQUAD = H // 4       # 4 head quads
SCALE = float(C) ** -0.5   # 1/32 applied inside exp

F32 = mybir.dt.float32
F32R = mybir.dt.float32r
BF16 = mybir.dt.bfloat16
AF = mybir.ActivationFunctionType
# matmul operand dtype for the QKV/ST/proj chains: F32R (TF32-like, exact-ish)
# or BF16 (explicit LDWEIGHTS, pipelined weight loads). Flip based on HW A/B.
USE_BF16 = True
MM_DT = BF16 if USE_BF16 else F32R
N_CORES = 8

_cache = {}


def _build():
    nc = bacc.Bacc("TRN2", target_bir_lowering=False, debug=False,
                   enable_asserts=False, num_devices=N_CORES)
    x = nc.dram_tensor("x", [T, C], F32, kind="ExternalInput").ap()
    wdt = F32 if USE_BF16 else F32R
    wq = nc.dram_tensor("wq", [H, C, D], wdt, kind="ExternalInput").ap()
    wk = nc.dram_tensor("wk", [H, C, D], wdt, kind="ExternalInput").ap()
    wv = nc.dram_tensor("wv", [H, C, D], wdt, kind="ExternalInput").ap()
    w_proj = nc.dram_tensor("w_proj", [C, C], wdt, kind="ExternalInput").ap()
    wdma = nc.gpsimd if USE_BF16 else nc.sync  # bf16 needs a casting DMA
    b_proj = nc.dram_tensor("b_proj", [C], F32, kind="ExternalInput").ap()
    out = nc.dram_tensor("out", [T, C], F32, kind="ExternalOutput").ap()

    with tile.TileContext(nc) as tc:
        with tc.tile_pool(name="dram", bufs=1, space="DRAM") as dram_pool, \
             tc.tile_pool(name="big", bufs=1) as big, \
             tc.tile_pool(name="st_ps", bufs=2, space="PSUM") as st_ps, \
             tc.tile_pool(name="work_ps", bufs=4, space="PSUM") as work_ps:

            if USE_BF16:
                ot_all = big.tile([P, NPAIR, T], BF16, tag="ot_all")
                ot_dram = None
            else:
                ot_all = None
                ot_dram = dram_pool.tile([NPAIR, P, T], MM_DT)

            ident = big.tile([P, P], F32, tag="ident")
            make_identity(nc, ident)
            ones64_f = big.tile([P, 64], F32, tag="ones64_f")
            nc.vector.memset(ones64_f, 1.0)
            ones64 = big.tile([P, 64], BF16, tag="ones64")
            nc.vector.tensor_copy(ones64, ones64_f)

            # ---------- Phase 0: xT [C, T] ----------
            xT = big.tile([P, KO, T], MM_DT, tag="xT")
            with tc.tile_pool(name="xin", bufs=2) as xin:
                for it in range(NT):
                    xtile = xin.tile([P, C], F32, tag="xtile")
                    nc.sync.dma_start(xtile, x[it * P:(it + 1) * P, :])
                    for ko in range(KO):
                        pt = work_ps.tile([P, 512], F32, tag="w")
                        nc.tensor.transpose(
                            pt[:, 0:P], xtile[:, ko * P:(ko + 1) * P], ident)
                        nc.vector.tensor_copy(
                            xT[:, ko, it * P:(it + 1) * P], pt[:, 0:P])

            # ---------- Phase 1: per-quad V, per-pair QKT + attention ----------
            with tc.tile_pool(name="qkt", bufs=1) as qkt, \
                 tc.tile_pool(name="vpool", bufs=1) as vpool, \
                 tc.tile_pool(name="wts", bufs=2) as wts, \
                 tc.tile_pool(name="ptp", bufs=6) as ptp, \
                 tc.tile_pool(name="small", bufs=3) as small:

                for o in range(2):
                    # V for 8 heads (one oct): v_sb[p, i, 64*h_local + d]
                    # N=512 matmuls amortize the fp32r self-weight-load.
                    wv_sb = wts.tile([P, KO, 512], MM_DT, tag="wv")
                    for hh in range(8):
                        wdma.dma_start(
                            wv_sb[:, :, hh * D:(hh + 1) * D],
                            wv[8 * o + hh].rearrange("(ko p) d -> p ko d", p=P))
                    v_sb = vpool.tile([P, NT, 512], BF16, tag="v")
                    for i in range(NT):
                        pv = work_ps.tile([P, 512], F32, tag="w")
                        for ko in range(KO):
                            nc.tensor.matmul(
                                pv, xT[:, ko, i * P:(i + 1) * P],
                                wv_sb[:, ko, :],
                                start=(ko == 0), stop=(ko == KO - 1))
                        nc.vector.tensor_copy(v_sb[:, i, :], pv)

                    for gg in range(4):
                        g = 4 * o + gg
                        hoff = 2 * gg * D  # col offset of this pair in v_sb

                        # -- QT / KT for the pair: [128 = 2 heads x 64, T] --
                        wq_sb = wts.tile([P, KO, P], MM_DT, tag="wq")
                        wk_sb = wts.tile([P, KO, P], MM_DT, tag="wk")
                        for hh in range(2):
                            wdma.dma_start(
                                wq_sb[:, :, hh * D:(hh + 1) * D],
                                wq[2 * g + hh].rearrange("(ko p) d -> p ko d", p=P))
                            wdma.dma_start(
                                wk_sb[:, :, hh * D:(hh + 1) * D],
                                wk[2 * g + hh].rearrange("(ko p) d -> p ko d", p=P))
                        qt = qkt.tile([P, T], MM_DT, tag="qt")
                        kt = qkt.tile([P, T], MM_DT, tag="kt")
                        for j in range(NJ):
                            pq = work_ps.tile([P, 512], F32, tag="w")
                            for ko in range(KO):
                                nc.tensor.matmul(
                                    pq, wq_sb[:, ko, :],
                                    xT[:, ko, j * 512:(j + 1) * 512],
                                    start=(ko == 0), stop=(ko == KO - 1))
                            nc.vector.tensor_copy(qt[:, j * 512:(j + 1) * 512], pq)
                            pk = work_ps.tile([P, 512], F32, tag="w")
                            for ko in range(KO):
                                nc.tensor.matmul(
                                    pk, wk_sb[:, ko, :],
                                    xT[:, ko, j * 512:(j + 1) * 512],
                                    start=(ko == 0), stop=(ko == KO - 1))
                            nc.vector.tensor_copy(kt[:, j * 512:(j + 1) * 512], pk)

                        # -- attention --
                        for j in range(NJ):
                            ot_ps = work_ps.tile([P, 512], F32, tag="w")
                            r_ps = work_ps.tile([P, 512], F32, tag="w")
                            n_i = 4 * j + 4
                            for i in range(n_i):
                                # diagonal tiles: only columns f >= lo are
                                # causally live; skip the dead left strip.
                                r = i - 4 * j
                                lo = P * r if r > 0 else 0
                                # fp32r matmul needs N>=256 for full rate
                                lo_st = lo if USE_BF16 else min(lo, 256)
                                st = st_ps.tile([P, 2, 512], F32, tag="st")
                                nc.tensor.matmul(
                                    st[:, 0, lo_st:],
                                    kt[0:64, i * P:(i + 1) * P],
                                    qt[0:64, j * 512 + lo_st:(j + 1) * 512],
                                    start=True, stop=True)
                                nc.tensor.matmul(
                                    st[:, 1, lo_st:],
                                    kt[64:128, i * P:(i + 1) * P],
                                    qt[64:128, j * 512 + lo_st:(j + 1) * 512],
                                    start=True, stop=True, tile_position=(64, 0))
                                pt = ptp.tile([P, 2, 512], BF16, tag="pt")
                                nc.scalar.activation(out=pt[:, :, lo:],
                                                     in_=st[:, :, lo:],
                                                     func=AF.Exp, scale=SCALE)
                                if r >= 0:  # diagonal tile: causal mask
                                    # keep where (lo + f_rel) - p - P*r >= 0
                                    nc.gpsimd.affine_select(
                                        out=pt[:, :, lo:], in_=pt[:, :, lo:],
                                        compare_op=mybir.AluOpType.is_ge,
                                        fill=0.0, base=0,
                                        channel_multiplier=-1,
                                        pattern=[[0, 2], [1, 512 - lo]])
                                first, last = (i == 0), (i == n_i - 1)
                                # O^T accumulation (col-tiled M=64 pair)
                                nc.tensor.matmul(
                                    ot_ps[0:64, lo:], v_sb[:, i, hoff:hoff + D],
                                    pt[:, 0, lo:], start=first, stop=last,
                                    tile_position=(0, 0))
                                nc.tensor.matmul(
                                    ot_ps[64:128, lo:],
                                    v_sb[:, i, hoff + D:hoff + 2 * D],
                                    pt[:, 1, lo:], start=first, stop=last,
                                    tile_position=(0, 64))
                                # row sums broadcast
                                nc.tensor.matmul(
                                    r_ps[0:64, lo:], ones64, pt[:, 0, lo:],
                                    start=first, stop=last, tile_position=(0, 0))
                                nc.tensor.matmul(
                                    r_ps[64:128, lo:], ones64, pt[:, 1, lo:],
                                    start=first, stop=last, tile_position=(0, 64))
                            recip = small.tile([P, 512], F32, tag="recip")
                            nc.vector.reciprocal(recip, r_ps)
                            if USE_BF16:
                                nc.vector.tensor_mul(
                                    ot_all[:, g, j * 512:(j + 1) * 512],
                                    ot_ps, recip)
                            else:
                                ot_sb = small.tile([P, 512], MM_DT, tag="ot_sb")
                                nc.vector.tensor_mul(ot_sb, ot_ps, recip)
                                nc.sync.dma_start(
                                    ot_dram[g, :, j * 512:(j + 1) * 512], ot_sb)

            # ---------- Phase 2: Y = OT.T @ w_proj + bias ----------
            with tc.tile_pool(name="proj", bufs=1) as proj, \
                 tc.tile_pool(name="otl", bufs=3) as otl, \
                 tc.tile_pool(name="yp", bufs=2) as yp:
                wp_sb = proj.tile([P, KO, C], MM_DT, tag="wp")
                wdma.dma_start(wp_sb, w_proj.rearrange("(ko p) c -> p ko c", p=P))
                bias_sb = proj.tile([P, C], F32, tag="bias")
                bias_bcast = bass.AP(
                    tensor=b_proj.tensor, offset=b_proj.offset,
                    ap=[[0, P]] + list(b_proj.ap))
                nc.gpsimd.dma_start(out=bias_sb, in_=bias_bcast)

                for it in range(NT):
                    if USE_BF16:
                        ot_t = ot_all[:, :, it * P:(it + 1) * P]
                    else:
                        ot_t = otl.tile([P, NPAIR, P], MM_DT, tag="ot_t")
                        nc.sync.dma_start(
                            ot_t,
                            ot_dram[:, :, it * P:(it + 1) * P]
                            .rearrange("g p t -> p g t"))
                    ysb = yp.tile([P, C], F32, tag="ysb")
                    for cc in range(2):
                        ypt = work_ps.tile([P, 512], F32, tag="w")
                        for g in range(NPAIR):
                            nc.tensor.matmul(
                                ypt, ot_t[:, g, :],
                                wp_sb[:, g, cc * 512:(cc + 1) * 512],
                                start=(g == 0), stop=(g == NPAIR - 1))
                        nc.vector.tensor_add(
                            ysb[:, cc * 512:(cc + 1) * 512], ypt,
                            bias_sb[:, cc * 512:(cc + 1) * 512])
                    nc.sync.dma_start(out[it * P:(it + 1) * P, :], ysb)

    nc.compile()
    return nc


def kernel(x, wq, wk, wv, w_proj, b_proj):
    x = np.ascontiguousarray(x, dtype=np.float32)
    wq = np.ascontiguousarray(wq, dtype=np.float32)
    wk = np.ascontiguousarray(wk, dtype=np.float32)
    wv = np.ascontiguousarray(wv, dtype=np.float32)
    w_proj = np.ascontiguousarray(w_proj, dtype=np.float32)
    b_proj = np.ascontiguousarray(b_proj, dtype=np.float32)

    if "nc" not in _cache:
        _cache["nc"] = _build()
    nc = _cache["nc"]

    in_maps = [
        {"x": x[b_], "wq": wq, "wk": wk, "wv": wv,
         "w_proj": w_proj, "b_proj": b_proj}
        for b_ in range(B)
    ]
    res = run_bass_kernel_spmd(nc, in_maps, core_ids=list(range(N_CORES)))
    return np.stack([res.results[b_]["out"] for b_ in range(B)], axis=0)


def run_traced(inputs, trace_cores=None):
    """Run with NTFF profiling; returns BassKernelResults (test-only helper)."""
    if "nc" not in _cache:
        _cache["nc"] = _build()
    nc = _cache["nc"]
    x = np.ascontiguousarray(inputs["x"], dtype=np.float32)
    in_maps = [
        {"x": x[b_],
         "wq": np.ascontiguousarray(inputs["wq"], dtype=np.float32),
         "wk": np.ascontiguousarray(inputs["wk"], dtype=np.float32),
         "wv": np.ascontiguousarray(inputs["wv"], dtype=np.float32),
         "w_proj": np.ascontiguousarray(inputs["w_proj"], dtype=np.float32),
         "b_proj": np.ascontiguousarray(inputs["b_proj"], dtype=np.float32)}
        for b_ in range(B)
    ]
    return run_bass_kernel_spmd(nc, in_maps, core_ids=list(range(N_CORES)),
                                trace=True, trace_cores=trace_cores)


if __name__ == "__main__":
    rng = np.random.default_rng(0)
    inputs = {
        "x": rng.standard_normal((B, T, C), dtype=np.float32),
        "wq": (rng.standard_normal((H, C, D), dtype=np.float32) * 0.02),
        "wk": (rng.standard_normal((H, C, D), dtype=np.float32) * 0.02),
        "wv": (rng.standard_normal((H, C, D), dtype=np.float32) * 0.02),
        "w_proj": (rng.standard_normal((C, C), dtype=np.float32) * 0.02),
        "b_proj": (rng.standard_normal((C,), dtype=np.float32) * 0.02),
    }
    y = kernel(**inputs)
    print("out", y.shape, y.dtype, np.abs(y).mean())


# revision 14
# speedup vs baseline: 1.0194x; 1.0194x over previous
"""Multi-head causal attention (B=8, T=2048, C=1024, H=16, D=64) on 8 TRN2 NeuronCores.

Strategy: pure data-parallel over batch (B=8 = n_cores, no collectives).
Each core processes one batch element:
  - transpose x -> xT [C, T] via PE (every C-contraction wants C on partitions
    for both operands)
  - per head-pair g (2 heads packed into 128 partitions):
      QT/KT [128, T] = w_pair.T @ xT     (heads stacked on partition dim)
      V     [s, 8*64] per head-oct       (8 heads packed on free dim, N=512)
      S^T tiles [s 128, tq 512] per head via row-tiled K=64 matmuls
        (tile_position (0,0)/(64,0): the two heads run concurrently on HW)
      P^T = exp(S^T / 32)  (ScalarE; no max-subtraction needed: |logits| < ~1,
        so exp cannot overflow and every row has its diagonal element)
      causal mask via gpsimd affine_select on diagonal tiles only; dead
        columns of diagonal tiles are never computed (lo strip skipping)
      O^T [d, tq] accumulated via col-tiled M=64 matmuls (lhsT = V, also
        concurrent via tile_position (0,0)/(0,64))
      row-sums broadcast to all partitions via ones-matmul (softmax denom),
      reciprocal + multiply folded into the PSUM->SBUF eviction of O^T
  - final projection Y = OT_all.T @ w_proj + bias, contiguous writeback

Matmul dtype: bf16 operands everywhere (USE_BF16=True; fp32r fallback kept).
HW-measured rel err vs float64 oracle: ~3.1e-3 (fp32r path: ~1.8e-3).
"""
import numpy as np

import concourse.bass as bass
import concourse.mybir as mybir
import concourse.tile as tile
from concourse import bacc
from concourse.bass_utils import run_bass_kernel_spmd
from concourse.masks import make_identity

B, T, C = 8, 2048, 1024
H, D = 16, 64
P = 128
KO = C // P          # 8 contraction chunks over C
NT = T // P          # 16 t-tiles of 128
NJ = T // 512        # 4 t-chunks of 512
NPAIR = H // 2       # 8 head pairs
NQUAD = H // 4       # 4 head quads
SCALE = float(C) ** -0.5   # 1/32 applied inside exp

F32 = mybir.dt.float32
F32R = mybir.dt.float32r
BF16 = mybir.dt.bfloat16
AF = mybir.ActivationFunctionType
# matmul operand dtype for the QKV/ST/proj chains: F32R (TF32-like, exact-ish)
# or BF16 (explicit LDWEIGHTS, pipelined weight loads). Flip based on HW A/B.
USE_BF16 = True
MM_DT = BF16 if USE_BF16 else F32R
N_CORES = 8

_cache = {}


def _build():
    nc = bacc.Bacc("TRN2", target_bir_lowering=False, debug=False,
                   enable_asserts=False, num_devices=N_CORES)
    x = nc.dram_tensor("x", [T, C], F32, kind="ExternalInput").ap()
    wdt = F32 if USE_BF16 else F32R
    wq = nc.dram_tensor("wq", [H, C, D], wdt, kind="ExternalInput").ap()
    wk = nc.dram_tensor("wk", [H, C, D], wdt, kind="ExternalInput").ap()
    wv = nc.dram_tensor("wv", [H, C, D], wdt, kind="ExternalInput").ap()
    w_proj = nc.dram_tensor("w_proj", [C, C], wdt, kind="ExternalInput").ap()
    wdma = nc.gpsimd if USE_BF16 else nc.sync  # bf16 needs a casting DMA
    b_proj = nc.dram_tensor("b_proj", [C], F32, kind="ExternalInput").ap()
    out = nc.dram_tensor("out", [T, C], F32, kind="ExternalOutput").ap()

    with tile.TileContext(nc) as tc:
        with tc.tile_pool(name="dram", bufs=1, space="DRAM") as dram_pool, \
             tc.tile_pool(name="big", bufs=1) as big, \
             tc.tile_pool(name="st_ps", bufs=2, space="PSUM") as st_ps, \
             tc.tile_pool(name="work_ps", bufs=4, space="PSUM") as work_ps:

            if USE_BF16:
                ot_all = big.tile([P, NPAIR, T], BF16, tag="ot_all")
                ot_dram = None
            else:
                ot_all = None
                ot_dram = dram_pool.tile([NPAIR, P, T], MM_DT)

            ident = big.tile([P, P], F32, tag="ident")
            make_identity(nc, ident)
            ones64_f = big.tile([P, 64], F32, tag="ones64_f")
            nc.vector.memset(ones64_f, 1.0)
            ones64 = big.tile([P, 64], BF16, tag="ones64")
            nc.vector.tensor_copy(ones64, ones64_f)

            # ---------- Phase 0: xT [C, T] ----------
            xT = big.tile([P, KO, T], MM_DT, tag="xT")
            with tc.tile_pool(name="xin", bufs=2) as xin:
                for it in range(NT):
                    xtile = xin.tile([P, C], F32, tag="xtile")
                    nc.sync.dma_start(xtile, x[it * P:(it + 1) * P, :])
                    for ko in range(KO):
                        pt = work_ps.tile([P, 512], F32, tag="w")
                        nc.tensor.transpose(
                            pt[:, 0:P], xtile[:, ko * P:(ko + 1) * P], ident)
                        nc.vector.tensor_copy(
                            xT[:, ko, it * P:(it + 1) * P], pt[:, 0:P])

            # ---------- Phase 1: per-quad V, per-pair QKT + attention ----------
            with tc.tile_pool(name="qkt", bufs=2) as qkt, \
                 tc.tile_pool(name="vpool", bufs=2) as vpool, \
                 tc.tile_pool(name="wts", bufs=2) as wts, \
                 tc.tile_pool(name="ptp", bufs=6) as ptp, \
                 tc.tile_pool(name="small", bufs=3) as small:

                for o in range(2):
                    # V for 8 heads (one oct): v_sb[p, i, 64*h_local + d]
                    # N=512 matmuls amortize the fp32r self-weight-load.
                    wv_sb = wts.tile([P, KO, 512], MM_DT, tag="wv")
                    for hh in range(8):
                        wdma.dma_start(
                            wv_sb[:, :, hh * D:(hh + 1) * D],
                            wv[8 * o + hh].rearrange("(ko p) d -> p ko d", p=P))
                    v_sb = vpool.tile([P, NT, 512], BF16, tag="v")
                    for i in range(NT):
                        pv = work_ps.tile([P, 512], F32, tag="w")
                        for ko in range(KO):
                            nc.tensor.matmul(
                                pv, xT[:, ko, i * P:(i + 1) * P],
                                wv_sb[:, ko, :],
                                start=(ko == 0), stop=(ko == KO - 1))
                        nc.vector.tensor_copy(v_sb[:, i, :], pv)

                    for gg in range(4):
                        g = 4 * o + gg
                        hoff = 2 * gg * D  # col offset of this pair in v_sb

                        # -- QT / KT for the pair: [128 = 2 heads x 64, T] --
                        wq_sb = wts.tile([P, KO, P], MM_DT, tag="wq")
                        wk_sb = wts.tile([P, KO, P], MM_DT, tag="wk")
                        for hh in range(2):
                            wdma.dma_start(
                                wq_sb[:, :, hh * D:(hh + 1) * D],
                                wq[2 * g + hh].rearrange("(ko p) d -> p ko d", p=P))
                            wdma.dma_start(
                                wk_sb[:, :, hh * D:(hh + 1) * D],
                                wk[2 * g + hh].rearrange("(ko p) d -> p ko d", p=P))
                        qt = qkt.tile([P, T], MM_DT, tag="qt")
                        kt = qkt.tile([P, T], MM_DT, tag="kt")
                        for j in range(NJ):
                            pq = work_ps.tile([P, 512], F32, tag="w")
                            for ko in range(KO):
                                nc.tensor.matmul(
                                    pq, wq_sb[:, ko, :],
                                    xT[:, ko, j * 512:(j + 1) * 512],
                                    start=(ko == 0), stop=(ko == KO - 1))
                            nc.vector.tensor_copy(qt[:, j * 512:(j + 1) * 512], pq)
                            pk = work_ps.tile([P, 512], F32, tag="w")
                            for ko in range(KO):
                                nc.tensor.matmul(
                                    pk, wk_sb[:, ko, :],
                                    xT[:, ko, j * 512:(j + 1) * 512],
                                    start=(ko == 0), stop=(ko == KO - 1))
                            nc.vector.tensor_copy(kt[:, j * 512:(j + 1) * 512], pk)

                        # -- attention --
                        for j in range(NJ):
                            ot_ps = work_ps.tile([P, 512], F32, tag="w")
                            r_ps = work_ps.tile([P, 512], F32, tag="w")
                            n_i = 4 * j + 4
                            for i in range(n_i):
                                # diagonal tiles: only columns f >= lo are
                                # causally live; skip the dead left strip.
                                r = i - 4 * j
                                lo = P * r if r > 0 else 0
                                # fp32r matmul needs N>=256 for full rate
                                lo_st = lo if USE_BF16 else min(lo, 256)
                                st = st_ps.tile([P, 2, 512], F32, tag="st")
                                nc.tensor.matmul(
                                    st[:, 0, lo_st:],
                                    kt[0:64, i * P:(i + 1) * P],
                                    qt[0:64, j * 512 + lo_st:(j + 1) * 512],
                                    start=True, stop=True)
                                nc.tensor.matmul(
                                    st[:, 1, lo_st:],
                                    kt[64:128, i * P:(i + 1) * P],
                                    qt[64:128, j * 512 + lo_st:(j + 1) * 512],
                                    start=True, stop=True, tile_position=(64, 0))
                                pt = ptp.tile([P, 2, 512], BF16, tag="pt")
                                nc.scalar.activation(out=pt[:, :, lo:],
                                                     in_=st[:, :, lo:],
                                                     func=AF.Exp, scale=SCALE)
                                if r >= 0:  # diagonal tile: causal mask
                                    # keep where (lo + f_rel) - p - P*r >= 0
                                    nc.gpsimd.affine_select(
                                        out=pt[:, :, lo:], in_=pt[:, :, lo:],
                                        compare_op=mybir.AluOpType.is_ge,
                                        fill=0.0, base=0,
                                        channel_multiplier=-1,
                                        pattern=[[0, 2], [1, 512 - lo]])
                                first, last = (i == 0), (i == n_i - 1)
                                # O^T accumulation (col-tiled M=64 pair)
                                nc.tensor.matmul(
                                    ot_ps[0:64, lo:], v_sb[:, i, hoff:hoff + D],
                                    pt[:, 0, lo:], start=first, stop=last,
                                    tile_position=(0, 0))
                                nc.tensor.matmul(
                                    ot_ps[64:128, lo:],
                                    v_sb[:, i, hoff + D:hoff + 2 * D],
                                    pt[:, 1, lo:], start=first, stop=last,
                                    tile_position=(0, 64))
                                # row sums broadcast
                                nc.tensor.matmul(
                                    r_ps[0:64, lo:], ones64, pt[:, 0, lo:],
                                    start=first, stop=last, tile_position=(0, 0))
                                nc.tensor.matmul(
                                    r_ps[64:128, lo:], ones64, pt[:, 1, lo:],
                                    start=first, stop=last, tile_position=(0, 64))
                            recip = small.tile([P, 512], F32, tag="recip")
                            nc.vector.reciprocal(recip, r_ps)
                            if USE_BF16:
                                nc.vector.tensor_mul(
                                    ot_all[:, g, j * 512:(j + 1) * 512],
                                    ot_ps, recip)
                            else:
                                ot_sb = small.tile([P, 512], MM_DT, tag="ot_sb")
                                nc.vector.tensor_mul(ot_sb, ot_ps, recip)
                                nc.sync.dma_start(
                                    ot_dram[g, :, j * 512:(j + 1) * 512], ot_sb)

            # ---------- Phase 2: Y = OT.T @ w_proj + bias ----------
            with tc.tile_pool(name="proj", bufs=1) as proj, \
                 tc.tile_pool(name="otl", bufs=3) as otl, \
                 tc.tile_pool(name="yp", bufs=2) as yp:
                wp_sb = proj.tile([P, KO, C], MM_DT, tag="wp")
                wdma.dma_start(wp_sb, w_proj.rearrange("(ko p) c -> p ko c", p=P))
                bias_sb = proj.tile([P, C], F32, tag="bias")
                bias_bcast = bass.AP(
                    tensor=b_proj.tensor, offset=b_proj.offset,
                    ap=[[0, P]] + list(b_proj.ap))
                nc.gpsimd.dma_start(out=bias_sb, in_=bias_bcast)

                for it in range(NT):
                    if USE_BF16:
                        ot_t = ot_all[:, :, it * P:(it + 1) * P]
                    else:
                        ot_t = otl.tile([P, NPAIR, P], MM_DT, tag="ot_t")
                        nc.sync.dma_start(
                            ot_t,
                            ot_dram[:, :, it * P:(it + 1) * P]
                            .rearrange("g p t -> p g t"))
                    ysb = yp.tile([P, C], F32, tag="ysb")
                    for cc in range(2):
                        ypt = work_ps.tile([P, 512], F32, tag="w")
                        for g in range(NPAIR):
                            nc.tensor.matmul(
                                ypt, ot_t[:, g, :],
                                wp_sb[:, g, cc * 512:(cc + 1) * 512],
                                start=(g == 0), stop=(g == NPAIR - 1))
                        nc.vector.tensor_add(
                            ysb[:, cc * 512:(cc + 1) * 512], ypt,
                            bias_sb[:, cc * 512:(cc + 1) * 512])
                    nc.sync.dma_start(out[it * P:(it + 1) * P, :], ysb)

    nc.compile()
    return nc


def kernel(x, wq, wk, wv, w_proj, b_proj):
    x = np.ascontiguousarray(x, dtype=np.float32)
    wq = np.ascontiguousarray(wq, dtype=np.float32)
    wk = np.ascontiguousarray(wk, dtype=np.float32)
    wv = np.ascontiguousarray(wv, dtype=np.float32)
    w_proj = np.ascontiguousarray(w_proj, dtype=np.float32)
    b_proj = np.ascontiguousarray(b_proj, dtype=np.float32)

    if "nc" not in _cache:
        _cache["nc"] = _build()
    nc = _cache["nc"]

    in_maps = [
        {"x": x[b_], "wq": wq, "wk": wk, "wv": wv,
         "w_proj": w_proj, "b_proj": b_proj}
        for b_ in range(B)
    ]
    res = run_bass_kernel_spmd(nc, in_maps, core_ids=list(range(N_CORES)))
    return np.stack([res.results[b_]["out"] for b_ in range(B)], axis=0)


def run_traced(inputs, trace_cores=None):
    """Run with NTFF profiling; returns BassKernelResults (test-only helper)."""
    if "nc" not in _cache:
        _cache["nc"] = _build()
    nc = _cache["nc"]
    x = np.ascontiguousarray(inputs["x"], dtype=np.float32)
    in_maps = [
        {"x": x[b_],
         "wq": np.ascontiguousarray(inputs["wq"], dtype=np.float32),
         "wk": np.ascontiguousarray(inputs["wk"], dtype=np.float32),
         "wv": np.ascontiguousarray(inputs["wv"], dtype=np.float32),
         "w_proj": np.ascontiguousarray(inputs["w_proj"], dtype=np.float32),
         "b_proj": np.ascontiguousarray(inputs["b_proj"], dtype=np.float32)}
        for b_ in range(B)
    ]
    return run_bass_kernel_spmd(nc, in_maps, core_ids=list(range(N_CORES)),
                                trace=True, trace_cores=trace_cores)


if __name__ == "__main__":
    rng = np.random.default_rng(0)
    inputs = {
        "x": rng.standard_normal((B, T, C), dtype=np.float32),
        "wq": (rng.standard_normal((H, C, D), dtype=np.float32) * 0.02),
        "wk": (rng.standard_normal((H, C, D), dtype=np.float32) * 0.02),
        "wv": (rng.standard_normal((H, C, D), dtype=np.float32) * 0.02),
        "w_proj": (rng.standard_normal((C, C), dtype=np.float32) * 0.02),
        "b_proj": (rng.standard_normal((C,), dtype=np.float32) * 0.02),
    }
    y = kernel(**inputs)
    print("out", y.shape, y.dtype, np.abs(y).mean())


# revision 15
# speedup vs baseline: 1.0874x; 1.0667x over previous
"""Multi-head causal attention (B=8, T=2048, C=1024, H=16, D=64) on 8 TRN2 NeuronCores.

Strategy: pure data-parallel over batch (B=8 = n_cores, no collectives).
Each core processes one batch element:
  - transpose x -> xT [C, T] via PE (every C-contraction wants C on partitions
    for both operands)
  - per head-pair g (2 heads packed into 128 partitions):
      QT/KT [128, T] = w_pair.T @ xT     (heads stacked on partition dim)
      V     [s, 8*64] per head-oct       (8 heads packed on free dim, N=512)
      S^T tiles [s 128, tq 512] per head via row-tiled K=64 matmuls
        (tile_position (0,0)/(64,0): the two heads run concurrently on HW)
      P^T = exp(S^T / 32)  (ScalarE; no max-subtraction needed: |logits| < ~1,
        so exp cannot overflow and every row has its diagonal element)
      causal mask via gpsimd affine_select on diagonal tiles only; dead
        columns of diagonal tiles are never computed (lo strip skipping)
      O^T [d, tq] accumulated via col-tiled M=64 matmuls (lhsT = V, also
        concurrent via tile_position (0,0)/(0,64))
      row-sums broadcast to all partitions via ones-matmul (softmax denom),
      reciprocal + multiply folded into the PSUM->SBUF eviction of O^T
  - final projection Y = OT_all.T @ w_proj + bias, contiguous writeback

Matmul dtype: bf16 operands everywhere (USE_BF16=True; fp32r fallback kept).
HW-measured rel err vs float64 oracle: ~3.1e-3 (fp32r path: ~1.8e-3).
"""
import numpy as np

import concourse.bass as bass
import concourse.mybir as mybir
import concourse.tile as tile
from concourse import bacc
from concourse.bass_utils import run_bass_kernel_spmd
from concourse.masks import make_identity

B, T, C = 8, 2048, 1024
H, D = 16, 64
P = 128
KO = C // P          # 8 contraction chunks over C
NT = T // P          # 16 t-tiles of 128
NJ = T // 512        # 4 t-chunks of 512
NPAIR = H // 2       # 8 head pairs
NQUAD = H // 4       # 4 head quads
SCALE = float(C) ** -0.5   # 1/32 applied inside exp

F32 = mybir.dt.float32
F32R = mybir.dt.float32r
BF16 = mybir.dt.bfloat16
AF = mybir.ActivationFunctionType
# matmul operand dtype for the QKV/ST/proj chains: F32R (TF32-like, exact-ish)
# or BF16 (explicit LDWEIGHTS, pipelined weight loads). Flip based on HW A/B.
USE_BF16 = True
MM_DT = BF16 if USE_BF16 else F32R
N_CORES = 8

_cache = {}


def _build():
    nc = bacc.Bacc("TRN2", target_bir_lowering=False, debug=False,
                   enable_asserts=False, num_devices=N_CORES)
    x = nc.dram_tensor("x", [T, C], F32, kind="ExternalInput").ap()
    wdt = F32 if USE_BF16 else F32R
    wq = nc.dram_tensor("wq", [H, C, D], wdt, kind="ExternalInput").ap()
    wk = nc.dram_tensor("wk", [H, C, D], wdt, kind="ExternalInput").ap()
    wv = nc.dram_tensor("wv", [H, C, D], wdt, kind="ExternalInput").ap()
    w_proj = nc.dram_tensor("w_proj", [C, C], wdt, kind="ExternalInput").ap()
    wdma = nc.gpsimd if USE_BF16 else nc.sync  # bf16 needs a casting DMA
    b_proj = nc.dram_tensor("b_proj", [C], F32, kind="ExternalInput").ap()
    out = nc.dram_tensor("out", [T, C], F32, kind="ExternalOutput").ap()

    with tile.TileContext(nc) as tc:
        with tc.tile_pool(name="dram", bufs=1, space="DRAM") as dram_pool, \
             tc.tile_pool(name="big", bufs=1) as big, \
             tc.tile_pool(name="st_ps", bufs=2, space="PSUM") as st_ps, \
             tc.tile_pool(name="work_ps", bufs=4, space="PSUM") as work_ps:

            if USE_BF16:
                ot_all = big.tile([P, NPAIR, T], BF16, tag="ot_all")
                ot_dram = None
            else:
                ot_all = None
                ot_dram = dram_pool.tile([NPAIR, P, T], MM_DT)

            ident = big.tile([P, P], F32, tag="ident")
            make_identity(nc, ident)
            ones64_f = big.tile([P, 64], F32, tag="ones64_f")
            nc.vector.memset(ones64_f, 1.0)
            ones64 = big.tile([P, 64], BF16, tag="ones64")
            nc.vector.tensor_copy(ones64, ones64_f)

            # ---------- Phase 0: xT [C, T] ----------
            xT = big.tile([P, KO, T], MM_DT, tag="xT")
            with tc.tile_pool(name="xin", bufs=2) as xin:
                for it in range(NT):
                    xtile = xin.tile([P, C], F32, tag="xtile")
                    nc.sync.dma_start(xtile, x[it * P:(it + 1) * P, :])
                    for ko in range(KO):
                        pt = work_ps.tile([P, 512], F32, tag="w")
                        nc.tensor.transpose(
                            pt[:, 0:P], xtile[:, ko * P:(ko + 1) * P], ident)
                        nc.vector.tensor_copy(
                            xT[:, ko, it * P:(it + 1) * P], pt[:, 0:P])

            # ---------- Phase 1: per-quad V, per-pair QKT + attention ----------
            with tc.tile_pool(name="qkt", bufs=2) as qkt, \
                 tc.tile_pool(name="vpool", bufs=2) as vpool, \
                 tc.tile_pool(name="wts", bufs=2) as wts, \
                 tc.tile_pool(name="ptp", bufs=6) as ptp, \
                 tc.tile_pool(name="small", bufs=3) as small:

                for o in range(2):
                    # V for 8 heads (one oct): v_sb[p, i, 64*h_local + d]
                    # N=512 matmuls amortize the fp32r self-weight-load.
                    wv_sb = wts.tile([P, KO, 512], MM_DT, tag="wv")
                    for hh in range(8):
                        wdma.dma_start(
                            wv_sb[:, :, hh * D:(hh + 1) * D],
                            wv[8 * o + hh].rearrange("(ko p) d -> p ko d", p=P))
                    v_sb = vpool.tile([P, NT, 512], BF16, tag="v")
                    for i in range(NT):
                        pv = work_ps.tile([P, 512], F32, tag="w")
                        for ko in range(KO):
                            nc.tensor.matmul(
                                pv, xT[:, ko, i * P:(i + 1) * P],
                                wv_sb[:, ko, :],
                                start=(ko == 0), stop=(ko == KO - 1))
                        nc.vector.tensor_copy(v_sb[:, i, :], pv)

                    for gg in range(4):
                        g = 4 * o + gg
                        hoff = 2 * gg * D  # col offset of this pair in v_sb

                        # -- QT / KT for the pair: [128 = 2 heads x 64, T] --
                        wq_sb = wts.tile([P, KO, P], MM_DT, tag="wq")
                        wk_sb = wts.tile([P, KO, P], MM_DT, tag="wk")
                        for hh in range(2):
                            wdma.dma_start(
                                wq_sb[:, :, hh * D:(hh + 1) * D],
                                wq[2 * g + hh].rearrange("(ko p) d -> p ko d", p=P))
                            wdma.dma_start(
                                wk_sb[:, :, hh * D:(hh + 1) * D],
                                wk[2 * g + hh].rearrange("(ko p) d -> p ko d", p=P))
                        qt = qkt.tile([P, T], MM_DT, tag="qt")
                        kt = qkt.tile([P, T], MM_DT, tag="kt")
                        for j in range(NJ):
                            pq = work_ps.tile([P, 512], F32, tag="w")
                            for ko in range(KO):
                                nc.tensor.matmul(
                                    pq, wq_sb[:, ko, :],
                                    xT[:, ko, j * 512:(j + 1) * 512],
                                    start=(ko == 0), stop=(ko == KO - 1))
                            nc.vector.tensor_copy(qt[:, j * 512:(j + 1) * 512], pq)
                            pk = work_ps.tile([P, 512], F32, tag="w")
                            for ko in range(KO):
                                nc.tensor.matmul(
                                    pk, wk_sb[:, ko, :],
                                    xT[:, ko, j * 512:(j + 1) * 512],
                                    start=(ko == 0), stop=(ko == KO - 1))
                            nc.vector.tensor_copy(kt[:, j * 512:(j + 1) * 512], pk)

                        # -- attention --
                        # software-pipelined by one s-tile: emit ST/exp for
                        # tile i before OT/sums of tile i-1, so the in-order
                        # PE stream never waits on ACT's exp of the tile it
                        # is about to consume.
                        for j in range(NJ):
                            ot_ps = work_ps.tile([P, 512], F32, tag="w")
                            r_ps = work_ps.tile([P, 512], F32, tag="w")
                            n_i = 4 * j + 4
                            pts = {}

                            def lo_of(i):
                                r = i - 4 * j
                                return P * r if r > 0 else 0

                            for i in range(n_i + 1):
                                if i < n_i:
                                    # diagonal tiles: only columns f >= lo
                                    # are causally live; skip the dead strip.
                                    lo = lo_of(i)
                                    lo_st = lo if USE_BF16 else min(lo, 256)
                                    st = st_ps.tile([P, 2, 512], F32, tag="st")
                                    nc.tensor.matmul(
                                        st[:, 0, lo_st:],
                                        kt[0:64, i * P:(i + 1) * P],
                                        qt[0:64, j * 512 + lo_st:(j + 1) * 512],
                                        start=True, stop=True)
                                    nc.tensor.matmul(
                                        st[:, 1, lo_st:],
                                        kt[64:128, i * P:(i + 1) * P],
                                        qt[64:128, j * 512 + lo_st:(j + 1) * 512],
                                        start=True, stop=True,
                                        tile_position=(64, 0))
                                    pt = ptp.tile([P, 2, 512], BF16, tag="pt")
                                    nc.scalar.activation(out=pt[:, :, lo:],
                                                         in_=st[:, :, lo:],
                                                         func=AF.Exp, scale=SCALE)
                                    if i >= 4 * j:  # diagonal: causal mask
                                        # keep where (lo + f_rel) - p - lo >= 0
                                        nc.gpsimd.affine_select(
                                            out=pt[:, :, lo:], in_=pt[:, :, lo:],
                                            compare_op=mybir.AluOpType.is_ge,
                                            fill=0.0, base=0,
                                            channel_multiplier=-1,
                                            pattern=[[0, 2], [1, 512 - lo]])
                                    pts[i] = pt
                                if i >= 1:
                                    ii = i - 1
                                    lo = lo_of(ii)
                                    pt = pts.pop(ii)
                                    first, last = (ii == 0), (ii == n_i - 1)
                                    # O^T accumulation (col-tiled M=64 pair)
                                    nc.tensor.matmul(
                                        ot_ps[0:64, lo:],
                                        v_sb[:, ii, hoff:hoff + D],
                                        pt[:, 0, lo:], start=first, stop=last,
                                        tile_position=(0, 0))
                                    nc.tensor.matmul(
                                        ot_ps[64:128, lo:],
                                        v_sb[:, ii, hoff + D:hoff + 2 * D],
                                        pt[:, 1, lo:], start=first, stop=last,
                                        tile_position=(0, 64))
                                    # row sums broadcast
                                    nc.tensor.matmul(
                                        r_ps[0:64, lo:], ones64, pt[:, 0, lo:],
                                        start=first, stop=last,
                                        tile_position=(0, 0))
                                    nc.tensor.matmul(
                                        r_ps[64:128, lo:], ones64, pt[:, 1, lo:],
                                        start=first, stop=last,
                                        tile_position=(0, 64))
                            recip = small.tile([P, 512], F32, tag="recip")
                            nc.vector.reciprocal(recip, r_ps)
                            if USE_BF16:
                                nc.vector.tensor_mul(
                                    ot_all[:, g, j * 512:(j + 1) * 512],
                                    ot_ps, recip)
                            else:
                                ot_sb = small.tile([P, 512], MM_DT, tag="ot_sb")
                                nc.vector.tensor_mul(ot_sb, ot_ps, recip)
                                nc.sync.dma_start(
                                    ot_dram[g, :, j * 512:(j + 1) * 512], ot_sb)

            # ---------- Phase 2: Y = OT.T @ w_proj + bias ----------
            with tc.tile_pool(name="proj", bufs=1) as proj, \
                 tc.tile_pool(name="otl", bufs=3) as otl, \
                 tc.tile_pool(name="yp", bufs=2) as yp:
                wp_sb = proj.tile([P, KO, C], MM_DT, tag="wp")
                wdma.dma_start(wp_sb, w_proj.rearrange("(ko p) c -> p ko c", p=P))
                bias_sb = proj.tile([P, C], F32, tag="bias")
                bias_bcast = bass.AP(
                    tensor=b_proj.tensor, offset=b_proj.offset,
                    ap=[[0, P]] + list(b_proj.ap))
                nc.gpsimd.dma_start(out=bias_sb, in_=bias_bcast)

                for it in range(NT):
                    if USE_BF16:
                        ot_t = ot_all[:, :, it * P:(it + 1) * P]
                    else:
                        ot_t = otl.tile([P, NPAIR, P], MM_DT, tag="ot_t")
                        nc.sync.dma_start(
                            ot_t,
                            ot_dram[:, :, it * P:(it + 1) * P]
                            .rearrange("g p t -> p g t"))
                    ysb = yp.tile([P, C], F32, tag="ysb")
                    for cc in range(2):
                        ypt = work_ps.tile([P, 512], F32, tag="w")
                        for g in range(NPAIR):
                            nc.tensor.matmul(
                                ypt, ot_t[:, g, :],
                                wp_sb[:, g, cc * 512:(cc + 1) * 512],
                                start=(g == 0), stop=(g == NPAIR - 1))
                        nc.vector.tensor_add(
                            ysb[:, cc * 512:(cc + 1) * 512], ypt,
                            bias_sb[:, cc * 512:(cc + 1) * 512])
                    nc.sync.dma_start(out[it * P:(it + 1) * P, :], ysb)

    nc.compile()
    return nc


def kernel(x, wq, wk, wv, w_proj, b_proj):
    x = np.ascontiguousarray(x, dtype=np.float32)
    wq = np.ascontiguousarray(wq, dtype=np.float32)
    wk = np.ascontiguousarray(wk, dtype=np.float32)
    wv = np.ascontiguousarray(wv, dtype=np.float32)
    w_proj = np.ascontiguousarray(w_proj, dtype=np.float32)
    b_proj = np.ascontiguousarray(b_proj, dtype=np.float32)

    if "nc" not in _cache:
        _cache["nc"] = _build()
    nc = _cache["nc"]

    in_maps = [
        {"x": x[b_], "wq": wq, "wk": wk, "wv": wv,
         "w_proj": w_proj, "b_proj": b_proj}
        for b_ in range(B)
    ]
    res = run_bass_kernel_spmd(nc, in_maps, core_ids=list(range(N_CORES)))
    return np.stack([res.results[b_]["out"] for b_ in range(B)], axis=0)


def run_traced(inputs, trace_cores=None):
    """Run with NTFF profiling; returns BassKernelResults (test-only helper)."""
    if "nc" not in _cache:
        _cache["nc"] = _build()
    nc = _cache["nc"]
    x = np.ascontiguousarray(inputs["x"], dtype=np.float32)
    in_maps = [
        {"x": x[b_],
         "wq": np.ascontiguousarray(inputs["wq"], dtype=np.float32),
         "wk": np.ascontiguousarray(inputs["wk"], dtype=np.float32),
         "wv": np.ascontiguousarray(inputs["wv"], dtype=np.float32),
         "w_proj": np.ascontiguousarray(inputs["w_proj"], dtype=np.float32),
         "b_proj": np.ascontiguousarray(inputs["b_proj"], dtype=np.float32)}
        for b_ in range(B)
    ]
    return run_bass_kernel_spmd(nc, in_maps, core_ids=list(range(N_CORES)),
                                trace=True, trace_cores=trace_cores)


if __name__ == "__main__":
    rng = np.random.default_rng(0)
    inputs = {
        "x": rng.standard_normal((B, T, C), dtype=np.float32),
        "wq": (rng.standard_normal((H, C, D), dtype=np.float32) * 0.02),
        "wk": (rng.standard_normal((H, C, D), dtype=np.float32) * 0.02),
        "wv": (rng.standard_normal((H, C, D), dtype=np.float32) * 0.02),
        "w_proj": (rng.standard_normal((C, C), dtype=np.float32) * 0.02),
        "b_proj": (rng.standard_normal((C,), dtype=np.float32) * 0.02),
    }
    y = kernel(**inputs)
    print("out", y.shape, y.dtype, np.abs(y).mean())
